# revision 41
# baseline (speedup 1.0000x reference)
"""MiniMind GQA attention block on 8 trn2 NeuronCores.

Sharding (per the TP-by-head hint): core c = (d, g) with d = c // 4 the
batch index (data parallel) and g = c % 4 the KV group (tensor parallel
over heads).  Each core computes q/k/v projections for its 4 query heads
and 1 KV head, RoPE, causal attention, and its o-projection partial
through its 256 rows of Wo; the host sums the four TP partials per batch
while unsharding (an on-device ReduceScatter has a ~20us latency floor
per call, and the final summation is ~0.1% of the FLOPs).

All matmul operands are bf16 (fp32 PSUM accumulate); everything on-chip
runs transposed (feature dims on partitions) so the softmax denominator
folds into the PV matmul via a v|ones stationary operand and no
probability transpose is ever needed.  The attention inner loop is
software-pipelined: the QK matmuls of group g+1 are emitted before the
PV matmuls of group g so the PE never waits on the scalar-engine exp,
each head's normalization broadcast rides two groups behind its DVE
reciprocal chain, and ~48 throwaway matmuls at kernel start hold the PE
activity monitor open so real work starts at 2.4 GHz.
"""

import numpy as np
from contextlib import ExitStack

B, S, H = 2, 2048, 1024
NH, NKV, HD = 16, 4, 64
P = 128
NCH = 4                # 512-wide sequence chunks
CHW = S // NCH         # 512
NCORES = 8

_prog_cache = {}


def _build():
    import concourse.bacc as bacc
    import concourse.mybir as mybir
    from concourse import tile

    F32 = mybir.dt.float32
    BF16 = mybir.dt.bfloat16
    EXP = mybir.ActivationFunctionType.Exp
    CPY = mybir.ActivationFunctionType.Copy
    MUL = mybir.AluOpType.mult
    ADD = mybir.AluOpType.add

    nc = bacc.Bacc()

    xT = nc.declare_dram_parameter("xT", [128, 8, S], BF16, isOutput=False)
    wq = nc.declare_dram_parameter("wq", [128, 8, 256], BF16, isOutput=False)
    wkv = nc.declare_dram_parameter("wkv", [128, 8, 128], BF16,
                                    isOutput=False)
    wo = nc.declare_dram_parameter("wo", [128, 2, H], BF16, isOutput=False)
    ct2 = nc.declare_dram_parameter("ct2", [128, S], BF16, isOutput=False)
    st2 = nc.declare_dram_parameter("st2", [128, S], BF16, isOutput=False)
    rot = nc.declare_dram_parameter("rot", [128, 128], BF16, isOutput=False)
    ident = nc.declare_dram_parameter("ident", [64, 64], BF16, isOutput=False)
    tri = nc.declare_dram_parameter("tri", [128, 128], BF16, isOutput=False)
    ones1 = nc.declare_dram_parameter("ones1", [1, 64], BF16, isOutput=False)
    onescol = nc.declare_dram_parameter("onescol", [128, 1], BF16,
                                        isOutput=False)
    out = nc.declare_dram_parameter("out", [S, H], BF16, isOutput=True)

    with ExitStack() as ctx:
        tc = ctx.enter_context(tile.TileContext(nc))
        ctx.enter_context(nc.allow_low_precision(reason="bf16 pipeline"))

        const = ctx.enter_context(tc.tile_pool(name="const", bufs=1))
        xpool = ctx.enter_context(tc.tile_pool(name="xpool", bufs=2))
        wpool = ctx.enter_context(tc.tile_pool(name="wpool", bufs=1))
        qkv = ctx.enter_context(tc.tile_pool(name="qkv", bufs=1))
        work = ctx.enter_context(tc.tile_pool(name="work", bufs=4))
        probs_pool = ctx.enter_context(tc.tile_pool(name="probs_pool", bufs=3))
        attn_pool = ctx.enter_context(tc.tile_pool(name="attn_pool", bufs=2))
        obuf = ctx.enter_context(tc.tile_pool(name="obuf", bufs=3))

        # PSUM budget: pp 2x1 + sp 2x2 + vp 2x1 = 8 banks
        pp = ctx.enter_context(tc.tile_pool(name="pp", bufs=2, space="PSUM"))
        sp = ctx.enter_context(tc.tile_pool(name="sp", bufs=2, space="PSUM"))
        vp = ctx.enter_context(tc.tile_pool(name="vp", bufs=2, space="PSUM"))

        # ---- constants & weights to SBUF (DMA priority order: the first
        # kv-projection matmul only needs wkv + xc0) ----
        rot_t = const.tile([128, 128], BF16)
        ident_t = const.tile([64, 64], BF16)
        tri_t = const.tile([128, 128], BF16)
        ones1_t = const.tile([1, 64], BF16)
        onescol_t = const.tile([128, 1], BF16)
        ct2_t = const.tile([128, S], BF16)
        st2_t = const.tile([128, S], BF16)
        nc.sync.dma_start(rot_t[:], rot[:])
        xcb0 = xpool.tile([P, 8 * CHW], BF16, name="xcb")
        nc.sync.dma_start(xcb0[:], xT[:, :, 0:CHW])
        wkv_b = wpool.tile([P, 8 * 128], BF16, name="wkv_b")
        nc.sync.dma_start(wkv_b[:], wkv[:])
        nc.sync.dma_start(ident_t[:], ident[:])
        nc.sync.dma_start(tri_t[:], tri[:])
        nc.sync.dma_start(ones1_t[:], ones1[:])
        nc.sync.dma_start(onescol_t[:], onescol[:])
        wq_b = wpool.tile([P, 8 * 256], BF16, name="wq_b")
        wo_b = wpool.tile([P, 2 * H], BF16, name="wo_b")

        # ~48 throwaway matmuls on the first-arrived tile keep the PE
        # activity monitor busy while the x DMA lands, so the real
        # pipeline starts at the full 2.4 GHz clock instead of 1.2.
        warm = pp.tile([P, 128], F32, name="warm", tag="pj")
        for _ in range(48):
            nc.tensor.matmul(warm[:], rot_t[:], rot_t[:],
                             start=True, stop=True)
        warm_s = work.tile([P, 128], BF16, name="warms")
        nc.vector.tensor_copy(warm_s[:], warm[:])

        # ---- persistent intermediates ----
        # qT: one [128, S] tile per head pair (rows 0-63 head 2p, 64-127
        # head 2p+1); kT2: k^T duplicated into both halves (odd heads use
        # base=64 APs); v_aug per seq tile: cols 0-63 v, col 64 ones.
        qT = [qkv.tile([P, S], BF16, name=f"qT{p}") for p in range(2)]
        kT2 = qkv.tile([P, S], BF16)
        v_aug = [qkv.tile([P, 66], BF16, name=f"vaug{t}")
                 for t in range(S // P)]

        def emit_proj(n):
            """Projections + RoPE for chunk n (DVE work all-bf16)."""
            cs = slice(n * CHW, (n + 1) * CHW)
            if n == 0:
                xcb = xcb0
            else:
                xcb = xpool.tile([P, 8 * CHW], BF16, name="xcb")
                nc.sync.dma_start(xcb[:], xT[:, :, cs])
            xc = [xcb[:, k * CHW:(k + 1) * CHW] for k in range(8)]
            if n == 0:
                nc.sync.dma_start(wq_b[:], wq[:])
                nc.sync.dma_start(ct2_t[:], ct2[:])
                nc.sync.dma_start(st2_t[:], st2[:])

            # kv projection (k rows 0-63, v rows 64-127).  Emission order
            # keeps every pp-arena reuse >= 2 matmul-groups behind the DVE
            # copy that releases it, so the PE never stalls.
            kvp = pp.tile([P, CHW], F32, name="kvp", tag="pj")
            for kt in range(8):
                nc.tensor.matmul(kvp[:], wkv_b[:, kt * 128:(kt + 1) * 128],
                                 xc[kt],
                                 start=(kt == 0), stop=(kt == 7))
            kraw = work.tile([64, CHW], BF16, name="kraw")
            nc.vector.tensor_copy(kraw[:], kvp[0:64, :])
            vTs = work.tile([64, CHW], BF16, name="vTs")
            nc.vector.tensor_copy(vTs[:], kvp[64:128, :])
            qp0 = pp.tile([P, CHW], F32, name="qp0", tag="pj")
            for kt in range(8):
                nc.tensor.matmul(qp0[:], wq_b[:, kt * 256:kt * 256 + P],
                                 xc[kt], start=(kt == 0), stop=(kt == 7))
            qraw0 = work.tile([P, CHW], BF16, name="qraw0")
            nc.vector.tensor_copy(qraw0[:], qp0[:])
            krp = pp.tile([64, CHW], F32, name="krp", tag="pj")
            nc.tensor.matmul(krp[:], rot_t[0:64, 0:64], kraw[:],
                             start=True, stop=True)
            krb = work.tile([64, CHW], BF16, name="krb")
            nc.vector.tensor_copy(krb[:], krp[:])
            qp1 = pp.tile([P, CHW], F32, name="qp1", tag="pj")
            for kt in range(8):
                nc.tensor.matmul(qp1[:],
                                 wq_b[:, kt * 256 + P:(kt + 1) * 256],
                                 xc[kt], start=(kt == 0), stop=(kt == 7))
            qraw1 = work.tile([P, CHW], BF16, name="qraw1")
            nc.vector.tensor_copy(qraw1[:], qp1[:])
            qraw = [qraw0, qraw1]
            rpb = []
            for pr in range(2):
                rp = pp.tile([P, CHW], F32, name=f"rp{pr}", tag="pj")
                nc.tensor.matmul(rp[:], rot_t[:], qraw[pr][:], start=True,
                                 stop=True)
                rb = work.tile([P, CHW], BF16, name=f"rpb{pr}")
                nc.vector.tensor_copy(rb[:], rp[:])
                rpb.append(rb)
            # v transpose per 128-block
            for j in range(CHW // P):
                t = n * (CHW // P) + j
                tp = pp.tile([P, 64], BF16, name="tp", tag="pj")
                nc.tensor.transpose(tp[:], vTs[:, j * P:(j + 1) * P],
                                    ident_t[:])
                nc.vector.tensor_copy(v_aug[t][:, 0:64], tp[:])
                nc.vector.tensor_copy(v_aug[t][:, 64:65], onescol_t[:])
            # k RoPE on the bf16 fast path
            ktm1 = work.tile([64, CHW], BF16, name="ktm1")
            nc.vector.tensor_tensor(ktm1[:], kraw[:], ct2_t[0:64, cs], MUL)
            ktm2 = work.tile([64, CHW], BF16, name="ktm2")
            nc.vector.tensor_tensor(ktm2[:], krb[:], st2_t[0:64, cs], MUL)
            nc.vector.tensor_tensor(kT2[0:64, cs], ktm2[:], ktm1[:], ADD)
            nc.vector.tensor_copy(kT2[64:128, cs], kT2[0:64, cs])
            # q RoPE
            for pr in range(2):
                tmp1 = work.tile([P, CHW], BF16, name=f"tmp1{pr}")
                nc.vector.tensor_tensor(tmp1[:], qraw[pr][:], ct2_t[:, cs],
                                        MUL)
                tmp2 = work.tile([P, CHW], BF16, name=f"tmp2{pr}")
                nc.vector.tensor_tensor(tmp2[:], rpb[pr][:], st2_t[:, cs],
                                        MUL)
                nc.vector.tensor_tensor(qT[pr][:, cs], tmp2[:], tmp1[:],
                                        ADD)
            if n == 0:
                nc.sync.dma_start(wo_b[:], wo[:])

        def emit_attention(n, at_c):
            """Attention for chunk n, 4 heads, flat software pipeline."""
            base = n * CHW
            # flat group list: (h, kind, arg)
            groups = []
            for h in range(4):
                for fg in range(2 * n):
                    groups.append((h, "full", 2 * fg))
                groups.append((h, "diag", 0))
                groups.append((h, "diag", 2))

            state = {}   # h -> dict(pv=..., first=...)
            pend = []    # emitted-QK groups awaiting exp/PV
            norm_a = []  # heads awaiting stage-A norm (after last PV)
            norm_b = []  # (h, it) awaiting stage-B norm (bc + at-mul)

            def qk(i):
                h, kind, arg = groups[i]
                pr, off = h // 2, (h % 2) * 64
                sc = sp.tile([P, 2 * CHW], F32, name="sc", tag="sc")
                if kind == "full":
                    for u in range(2):
                        sk = arg + u
                        nc.tensor.matmul(
                            sc[:, u * CHW:(u + 1) * CHW],
                            kT2[off:off + 64, sk * P:(sk + 1) * P],
                            qT[pr][off:off + 64, base:base + CHW],
                            start=True, stop=True)
                else:
                    o = 0
                    for j in (arg, arg + 1):
                        w = CHW - j * P
                        sk = 4 * n + j
                        nc.tensor.matmul(
                            sc[:, o:o + w],
                            kT2[off:off + 64, sk * P:(sk + 1) * P],
                            qT[pr][off:off + 64, base + j * P:base + CHW],
                            start=True, stop=True)
                        o += w
                pend.append((i, sc))

            def flush_b(before=None):
                while norm_b and (before is None or norm_b[0][1] < before):
                    h, _ = norm_b.pop(0)
                    st = state[h]
                    pr, off = h // 2, (h % 2) * 64
                    bc = vp.tile([64, CHW], F32, name="bc", tag="pv")
                    nc.tensor.matmul(bc[:], ones1_t[:], st["rcp_b"][:],
                                     start=True, stop=True)
                    nc.vector.tensor_tensor(at_c[pr][off:off + 64, :],
                                            st["un"][0:64, :], bc[:], MUL)

            def exp_pv():
                i, sc = pend.pop(0)
                h, kind, arg = groups[i]
                if h not in state:
                    state[h] = {"pv": vp.tile([65, CHW], F32, name="pv",
                                              tag="pv"),
                                "first": True}
                st = state[h]
                pv = st["pv"]
                pb = probs_pool.tile([P, 2 * CHW], BF16, name="pb")
                last_of_head = (i + 1 >= len(groups) or groups[i + 1][0] != h)
                if kind == "full":
                    nc.scalar.activation(pb[:], sc[:], EXP, scale=0.125)
                    for u in range(2):
                        sk = arg + u
                        nc.tensor.matmul(pv[:], v_aug[sk][:, 0:65],
                                         pb[:, u * CHW:(u + 1) * CHW],
                                         start=st["first"], stop=False)
                        st["first"] = False
                else:
                    o = sum(CHW - j * P for j in (arg, arg + 1))
                    nc.scalar.activation(pb[:, 0:o], sc[:, 0:o], EXP,
                                         scale=0.125)
                    o = 0
                    for j in (arg, arg + 1):
                        w = CHW - j * P
                        sk = 4 * n + j
                        nc.gpsimd.tensor_tensor(pb[:, o:o + P],
                                                pb[:, o:o + P], tri_t[:], MUL)
                        stop = last_of_head and j == arg + 1
                        nc.tensor.matmul(pv[:, j * P:CHW],
                                         v_aug[sk][:, 0:65], pb[:, o:o + w],
                                         start=st["first"], stop=stop)
                        st["first"] = False
                        o += w
                if last_of_head:
                    norm_a.append(h)

            def flush_a(it):
                while norm_a:
                    h = norm_a.pop(0)
                    st = state[h]
                    un = work.tile([65, CHW], F32, name="un")
                    nc.scalar.activation(un[:], st["pv"][0:65, :], CPY)
                    # the custom-DVE reciprocal mishandles partition
                    # offsets on hw: stage the denominator at partition 0
                    den = work.tile([1, CHW], F32, name="den")
                    nc.vector.tensor_copy(den[:], un[64:65, :])
                    rcp = work.tile([1, CHW], F32, name="rcp")
                    nc.vector.reciprocal_approx_fast(rcp[:], den[:])
                    rcp_b = work.tile([1, CHW], BF16, name="rcpb")
                    nc.vector.tensor_copy(rcp_b[:], rcp[:])
                    st["un"], st["rcp_b"] = un, rcp_b
                    norm_b.append((h, it))

            for i in range(len(groups)):
                qk(i)
                # bc matmuls ride >=2 groups behind their DVE chain
                flush_b(before=i - 1)
                if i >= 1:
                    exp_pv()
                    flush_a(i)
            exp_pv()
            flush_a(len(groups))
            # last heads' stage-B flushed by the caller under the o-proj
            # kt=0 matmuls
            return flush_b

        def emit_oproj(n, at_c):
            # Each core ships its raw o-proj partial (its 256 Wo rows);
            # the host sums the four TP partials per batch.  On-device
            # ReduceScatter has a ~20us latency floor per call that is
            # hard to hide, and the summation is 0.1% of the FLOPs.
            for m in range(4):
                obm = obuf.tile([P, H], BF16, name="ob")
                for nh in range(2):
                    po = pp.tile([P, CHW], F32, name="po", tag="pj")
                    for kt in range(2):
                        nc.tensor.matmul(
                            po[:], at_c[kt][:, m * P:(m + 1) * P],
                            wo_b[:, kt * H + nh * CHW:kt * H + (nh + 1) * CHW],
                            start=(kt == 0), stop=(kt == 1))
                    if nh == 0:
                        nc.vector.tensor_copy(obm[:, 0:CHW], po[:])
                    else:
                        nc.scalar.activation(obm[:, CHW:H], po[:], CPY)
                nc.gpsimd.dma_start(
                    out[n * CHW + m * P:n * CHW + (m + 1) * P, :], obm[:])

        emit_proj(0)
        for n in range(NCH):
            at_c = [attn_pool.tile([P, CHW], BF16, name=f"at_{kt}")
                    for kt in range(2)]
            fb = emit_attention(n, at_c)
            fb()
            if n + 1 < NCH:
                emit_proj(n + 1)
            emit_oproj(n, at_c)

    nc.compile()
    return nc


def _host_inputs(hidden_states, cos, sin, Wq, Wk, Wv, Wo):
    import ml_dtypes

    bf16 = ml_dtypes.bfloat16
    x = np.asarray(hidden_states, np.float32)
    cos = np.asarray(cos, np.float32)
    sin = np.asarray(sin, np.float32)
    Wq = np.asarray(Wq, np.float32)
    Wk = np.asarray(Wk, np.float32)
    Wv = np.asarray(Wv, np.float32)
    Wo = np.asarray(Wo, np.float32)

    ct2 = np.ascontiguousarray(np.tile(cos.T, (2, 1)))       # [128, S]
    st2 = np.ascontiguousarray(np.tile(sin.T, (2, 1)))
    r64 = np.zeros((64, 64), np.float32)
    for i in range(32):
        r64[32 + i, i] = -1.0
        r64[i, 32 + i] = 1.0
    rot = np.zeros((128, 128), np.float32)
    rot[0:64, 0:64] = r64
    rot[64:128, 64:128] = r64
    ident = np.eye(64, dtype=np.float32)
    tri = np.triu(np.ones((128, 128), np.float32))
    ones1 = np.ones((1, 64), np.float32)
    onescol = np.ones((128, 1), np.float32)

    def ktile(a, nt):
        # [nt*128, F] -> [128, nt, F] (partition-major interleave so one
        # DMA covers all nt contraction tiles)
        f = a.shape[1]
        return np.ascontiguousarray(
            a.reshape(nt, 128, f).transpose(1, 0, 2).astype(bf16))

    xTs = [ktile(x[d].T, 8) for d in range(B)]
    in_maps = []
    for c_id in range(NCORES):
        d, g = c_id // 4, c_id % 4
        in_maps.append({
            "xT": xTs[d],
            "wq": ktile(Wq[:, g * 256:(g + 1) * 256], 8),
            "wkv": ktile(
                np.concatenate([Wk[:, g * 64:(g + 1) * 64],
                                Wv[:, g * 64:(g + 1) * 64]], axis=1), 8),
            "wo": ktile(Wo[g * 256:(g + 1) * 256, :], 2),
            "ct2": ct2.astype(bf16), "st2": st2.astype(bf16),
            "rot": rot.astype(bf16), "ident": ident.astype(bf16),
            "tri": tri.astype(bf16), "ones1": ones1.astype(bf16),
            "onescol": onescol.astype(bf16),
        })
    return in_maps


def _assemble(results):
    full = np.empty((B, S, H), np.float32)
    for d in range(B):
        acc = np.zeros((S, H), np.float32)
        for g in range(4):
            acc += np.asarray(results[d * 4 + g]["out"]).astype(np.float32)
        full[d] = acc
    return full


def kernel(hidden_states, cos, sin, attention_mask, Wq, Wk, Wv, Wo):
    from concourse.bass_utils import run_bass_kernel_spmd
    if "nc" not in _prog_cache:
        _prog_cache["nc"] = _build()
    nc = _prog_cache["nc"]
    in_maps = _host_inputs(hidden_states, cos, sin, Wq, Wk, Wv, Wo)
    res = run_bass_kernel_spmd(nc, in_maps, list(range(NCORES)))
    return _assemble(res.results)


# revision 43
# speedup vs baseline: 1.1835x; 1.1835x over previous
"""MiniMind GQA attention block on 8 trn2 NeuronCores.

Sharding (per the TP-by-head hint): core c = (d, g) with d = c // 4 the
batch index (data parallel) and g = c % 4 the KV group (tensor parallel
over heads).  Each core computes q/k/v projections for its 4 query heads
and 1 KV head, RoPE, causal attention, and its o-projection partial
through its 256 rows of Wo; the host sums the four TP partials per batch
while unsharding (an on-device ReduceScatter has a ~20us latency floor
per call, and the final summation is ~0.1% of the FLOPs).

All matmul operands are bf16 (fp32 PSUM accumulate); everything on-chip
runs transposed (feature dims on partitions) so the softmax denominator
folds into the PV matmul via a v|ones stationary operand and no
probability transpose is ever needed.  The attention inner loop is
software-pipelined: the QK matmuls of group g+1 are emitted before the
PV matmuls of group g so the PE never waits on the scalar-engine exp,
each head's normalization broadcast rides two groups behind its DVE
reciprocal chain, and ~48 throwaway matmuls at kernel start hold the PE
activity monitor open so real work starts at 2.4 GHz.
"""

import numpy as np
from contextlib import ExitStack

B, S, H = 2, 2048, 1024
NH, NKV, HD = 16, 4, 64
P = 128
NCH = 4                # 512-wide sequence chunks
CHW = S // NCH         # 512
NCORES = 8

_prog_cache = {}


def _build():
    import concourse.bacc as bacc
    import concourse.mybir as mybir
    from concourse import tile

    F32 = mybir.dt.float32
    BF16 = mybir.dt.bfloat16
    EXP = mybir.ActivationFunctionType.Exp
    CPY = mybir.ActivationFunctionType.Copy
    MUL = mybir.AluOpType.mult
    ADD = mybir.AluOpType.add

    nc = bacc.Bacc()

    xT = nc.declare_dram_parameter("xT", [128, 8, S], BF16, isOutput=False)
    wq = nc.declare_dram_parameter("wq", [128, 8, 256], BF16, isOutput=False)
    wkv = nc.declare_dram_parameter("wkv", [128, 8, 128], BF16,
                                    isOutput=False)
    wo = nc.declare_dram_parameter("wo", [128, 2, H], BF16, isOutput=False)
    ct2 = nc.declare_dram_parameter("ct2", [128, S], BF16, isOutput=False)
    st2 = nc.declare_dram_parameter("st2", [128, S], BF16, isOutput=False)
    rot = nc.declare_dram_parameter("rot", [128, 128], BF16, isOutput=False)
    ident = nc.declare_dram_parameter("ident", [64, 64], BF16, isOutput=False)
    tri = nc.declare_dram_parameter("tri", [128, 128], BF16, isOutput=False)
    ones1 = nc.declare_dram_parameter("ones1", [1, 64], BF16, isOutput=False)
    onescol = nc.declare_dram_parameter("onescol", [128, 1], BF16,
                                        isOutput=False)
    out = nc.declare_dram_parameter("out", [S, H], BF16, isOutput=True)

    with ExitStack() as ctx:
        tc = ctx.enter_context(tile.TileContext(nc))
        ctx.enter_context(nc.allow_low_precision(reason="bf16 pipeline"))

        const = ctx.enter_context(tc.tile_pool(name="const", bufs=1))
        xpool = ctx.enter_context(tc.tile_pool(name="xpool", bufs=2))
        wpool = ctx.enter_context(tc.tile_pool(name="wpool", bufs=1))
        qkv = ctx.enter_context(tc.tile_pool(name="qkv", bufs=1))
        work = ctx.enter_context(tc.tile_pool(name="work", bufs=4))
        probs_pool = ctx.enter_context(tc.tile_pool(name="probs_pool", bufs=3))
        attn_pool = ctx.enter_context(tc.tile_pool(name="attn_pool", bufs=2))
        obuf = ctx.enter_context(tc.tile_pool(name="obuf", bufs=3))

        # PSUM budget: pp 2x1 + sp 2x2 + vp 2x1 = 8 banks
        pp = ctx.enter_context(tc.tile_pool(name="pp", bufs=2, space="PSUM"))
        sp = ctx.enter_context(tc.tile_pool(name="sp", bufs=2, space="PSUM"))
        vp = ctx.enter_context(tc.tile_pool(name="vp", bufs=2, space="PSUM"))

        # ---- constants & weights to SBUF (DMA priority order: the first
        # kv-projection matmul only needs wkv + xc0) ----
        rot_t = const.tile([128, 128], BF16)
        ident_t = const.tile([64, 64], BF16)
        tri_t = const.tile([128, 128], BF16)
        ones1_t = const.tile([1, 64], BF16)
        onescol_t = const.tile([128, 1], BF16)
        ct2_t = const.tile([128, S], BF16)
        st2_t = const.tile([128, S], BF16)
        nc.sync.dma_start(rot_t[:], rot[:])
        xcb0 = xpool.tile([P, 8 * CHW], BF16, name="xcb")
        nc.sync.dma_start(xcb0[:], xT[:, :, 0:CHW])
        wkv_b = wpool.tile([P, 8 * 128], BF16, name="wkv_b")
        nc.sync.dma_start(wkv_b[:], wkv[:])
        nc.sync.dma_start(ident_t[:], ident[:])
        nc.sync.dma_start(tri_t[:], tri[:])
        nc.sync.dma_start(ones1_t[:], ones1[:])
        nc.sync.dma_start(onescol_t[:], onescol[:])
        wq_b = wpool.tile([P, 8 * 256], BF16, name="wq_b")
        wo_b = wpool.tile([P, 2 * H], BF16, name="wo_b")

        # ~72 throwaway matmuls on the first-arrived tile keep the PE
        # activity monitor busy while the x DMA lands, so the real
        # pipeline starts at the full 2.4 GHz clock instead of 1.2.
        warm = pp.tile([P, 128], F32, name="warm", tag="pj")
        for _ in range(72):
            nc.tensor.matmul(warm[:], rot_t[:], rot_t[:],
                             start=True, stop=True)
        warm_s = work.tile([P, 128], BF16, name="warms")
        nc.vector.tensor_copy(warm_s[:], warm[:])

        # ---- persistent intermediates ----
        # qT: one [128, S] tile per head pair (rows 0-63 head 2p, 64-127
        # head 2p+1); kT2: k^T duplicated into both halves (odd heads use
        # base=64 APs); v_aug per seq tile: cols 0-63 v, col 64 ones.
        qT = [qkv.tile([P, S], BF16, name=f"qT{p}") for p in range(2)]
        kT2 = qkv.tile([P, S], BF16)
        v_aug = [qkv.tile([P, 66], BF16, name=f"vaug{t}")
                 for t in range(S // P)]

        def emit_proj(n):
            """Projections + RoPE for chunk n (DVE work all-bf16)."""
            cs = slice(n * CHW, (n + 1) * CHW)
            if n == 0:
                xcb = xcb0
            else:
                xcb = xpool.tile([P, 8 * CHW], BF16, name="xcb")
                nc.sync.dma_start(xcb[:], xT[:, :, cs])
            xc = [xcb[:, k * CHW:(k + 1) * CHW] for k in range(8)]
            if n == 0:
                nc.sync.dma_start(wq_b[:], wq[:])
                nc.sync.dma_start(ct2_t[:], ct2[:])
                nc.sync.dma_start(st2_t[:], st2[:])

            # kv projection (k rows 0-63, v rows 64-127).  Emission order
            # keeps every pp-arena reuse >= 2 matmul-groups behind the DVE
            # copy that releases it, so the PE never stalls.
            kvp = pp.tile([P, CHW], F32, name="kvp", tag="pj")
            for kt in range(8):
                nc.tensor.matmul(kvp[:], wkv_b[:, kt * 128:(kt + 1) * 128],
                                 xc[kt],
                                 start=(kt == 0), stop=(kt == 7))
            kraw = work.tile([64, CHW], BF16, name="kraw")
            nc.vector.tensor_copy(kraw[:], kvp[0:64, :])
            vTs = work.tile([64, CHW], BF16, name="vTs")
            nc.vector.tensor_copy(vTs[:], kvp[64:128, :])
            qp0 = pp.tile([P, CHW], F32, name="qp0", tag="pj")
            for kt in range(8):
                nc.tensor.matmul(qp0[:], wq_b[:, kt * 256:kt * 256 + P],
                                 xc[kt], start=(kt == 0), stop=(kt == 7))
            qraw0 = work.tile([P, CHW], BF16, name="qraw0")
            nc.vector.tensor_copy(qraw0[:], qp0[:])
            krp = pp.tile([64, CHW], F32, name="krp", tag="pj")
            nc.tensor.matmul(krp[:], rot_t[0:64, 0:64], kraw[:],
                             start=True, stop=True)
            krb = work.tile([64, CHW], BF16, name="krb")
            nc.vector.tensor_copy(krb[:], krp[:])
            qp1 = pp.tile([P, CHW], F32, name="qp1", tag="pj")
            for kt in range(8):
                nc.tensor.matmul(qp1[:],
                                 wq_b[:, kt * 256 + P:(kt + 1) * 256],
                                 xc[kt], start=(kt == 0), stop=(kt == 7))
            qraw1 = work.tile([P, CHW], BF16, name="qraw1")
            nc.vector.tensor_copy(qraw1[:], qp1[:])
            qraw = [qraw0, qraw1]
            rpb = []
            for pr in range(2):
                rp = pp.tile([P, CHW], F32, name=f"rp{pr}", tag="pj")
                nc.tensor.matmul(rp[:], rot_t[:], qraw[pr][:], start=True,
                                 stop=True)
                rb = work.tile([P, CHW], BF16, name=f"rpb{pr}")
                nc.vector.tensor_copy(rb[:], rp[:])
                rpb.append(rb)
            # v transpose per 128-block
            for j in range(CHW // P):
                t = n * (CHW // P) + j
                tp = pp.tile([P, 64], BF16, name="tp", tag="pj")
                nc.tensor.transpose(tp[:], vTs[:, j * P:(j + 1) * P],
                                    ident_t[:])
                nc.vector.tensor_copy(v_aug[t][:, 0:64], tp[:])
                nc.vector.tensor_copy(v_aug[t][:, 64:65], onescol_t[:])
            # k RoPE on the bf16 fast path
            ktm1 = work.tile([64, CHW], BF16, name="ktm1")
            nc.vector.tensor_tensor(ktm1[:], kraw[:], ct2_t[0:64, cs], MUL)
            ktm2 = work.tile([64, CHW], BF16, name="ktm2")
            nc.vector.tensor_tensor(ktm2[:], krb[:], st2_t[0:64, cs], MUL)
            nc.vector.tensor_tensor(kT2[0:64, cs], ktm2[:], ktm1[:], ADD)
            nc.vector.tensor_copy(kT2[64:128, cs], kT2[0:64, cs])
            # q RoPE
            for pr in range(2):
                tmp1 = work.tile([P, CHW], BF16, name=f"tmp1{pr}")
                nc.vector.tensor_tensor(tmp1[:], qraw[pr][:], ct2_t[:, cs],
                                        MUL)
                tmp2 = work.tile([P, CHW], BF16, name=f"tmp2{pr}")
                nc.vector.tensor_tensor(tmp2[:], rpb[pr][:], st2_t[:, cs],
                                        MUL)
                nc.vector.tensor_tensor(qT[pr][:, cs], tmp2[:], tmp1[:],
                                        ADD)
            if n == 0:
                nc.sync.dma_start(wo_b[:], wo[:])

        def emit_attention(n, at_c):
            """Attention for chunk n, 4 heads, flat software pipeline."""
            base = n * CHW
            # flat group list: (h, kind, arg)
            groups = []
            for h in range(4):
                for fg in range(2 * n):
                    groups.append((h, "full", 2 * fg))
                groups.append((h, "diag", 0))
                groups.append((h, "diag", 2))

            state = {}   # h -> dict(pv=..., first=...)
            pend = []    # emitted-QK groups awaiting exp/PV
            norm_a = []  # heads awaiting stage-A norm (after last PV)
            norm_b = []  # (h, it) awaiting stage-B norm (bc + at-mul)

            def qk(i):
                h, kind, arg = groups[i]
                pr, off = h // 2, (h % 2) * 64
                sc = sp.tile([P, 2 * CHW], F32, name="sc", tag="sc")
                if kind == "full":
                    for u in range(2):
                        sk = arg + u
                        nc.tensor.matmul(
                            sc[:, u * CHW:(u + 1) * CHW],
                            kT2[off:off + 64, sk * P:(sk + 1) * P],
                            qT[pr][off:off + 64, base:base + CHW],
                            start=True, stop=True)
                else:
                    o = 0
                    for j in (arg, arg + 1):
                        w = CHW - j * P
                        sk = 4 * n + j
                        nc.tensor.matmul(
                            sc[:, o:o + w],
                            kT2[off:off + 64, sk * P:(sk + 1) * P],
                            qT[pr][off:off + 64, base + j * P:base + CHW],
                            start=True, stop=True)
                        o += w
                pend.append((i, sc))

            def flush_b(before=None):
                while norm_b and (before is None or norm_b[0][1] < before):
                    h, _ = norm_b.pop(0)
                    st = state[h]
                    pr, off = h // 2, (h % 2) * 64
                    bc = vp.tile([64, CHW], F32, name="bc", tag="pv")
                    nc.tensor.matmul(bc[:], ones1_t[:], st["rcp_b"][:],
                                     start=True, stop=True)
                    nc.vector.tensor_tensor(at_c[pr][off:off + 64, :],
                                            st["un"][0:64, :], bc[:], MUL)

            def exp_pv():
                i, sc = pend.pop(0)
                h, kind, arg = groups[i]
                if h not in state:
                    state[h] = {"pv": vp.tile([65, CHW], F32, name="pv",
                                              tag="pv"),
                                "first": True}
                st = state[h]
                pv = st["pv"]
                pb = probs_pool.tile([P, 2 * CHW], BF16, name="pb")
                last_of_head = (i + 1 >= len(groups) or groups[i + 1][0] != h)
                if kind == "full":
                    nc.scalar.activation(pb[:], sc[:], EXP, scale=0.125)
                    for u in range(2):
                        sk = arg + u
                        nc.tensor.matmul(pv[:], v_aug[sk][:, 0:65],
                                         pb[:, u * CHW:(u + 1) * CHW],
                                         start=st["first"], stop=False)
                        st["first"] = False
                else:
                    o = sum(CHW - j * P for j in (arg, arg + 1))
                    nc.scalar.activation(pb[:, 0:o], sc[:, 0:o], EXP,
                                         scale=0.125)
                    o = 0
                    for j in (arg, arg + 1):
                        w = CHW - j * P
                        sk = 4 * n + j
                        nc.gpsimd.tensor_tensor(pb[:, o:o + P],
                                                pb[:, o:o + P], tri_t[:], MUL)
                        stop = last_of_head and j == arg + 1
                        nc.tensor.matmul(pv[:, j * P:CHW],
                                         v_aug[sk][:, 0:65], pb[:, o:o + w],
                                         start=st["first"], stop=stop)
                        st["first"] = False
                        o += w
                if last_of_head:
                    norm_a.append(h)

            def flush_a(it):
                while norm_a:
                    h = norm_a.pop(0)
                    st = state[h]
                    un = work.tile([65, CHW], F32, name="un")
                    nc.scalar.activation(un[:], st["pv"][0:65, :], CPY)
                    # the custom-DVE reciprocal mishandles partition
                    # offsets on hw: stage the denominator at partition 0
                    den = work.tile([1, CHW], F32, name="den")
                    nc.vector.tensor_copy(den[:], un[64:65, :])
                    rcp = work.tile([1, CHW], F32, name="rcp")
                    nc.vector.reciprocal_approx_fast(rcp[:], den[:])
                    rcp_b = work.tile([1, CHW], BF16, name="rcpb")
                    nc.vector.tensor_copy(rcp_b[:], rcp[:])
                    st["un"], st["rcp_b"] = un, rcp_b
                    norm_b.append((h, it))

            for i in range(len(groups)):
                qk(i)
                # bc matmuls ride >=2 groups behind their DVE chain
                flush_b(before=i - 1)
                if i >= 1:
                    exp_pv()
                    flush_a(i)
            exp_pv()
            flush_a(len(groups))
            # last heads' stage-B flushed by the caller under the o-proj
            # kt=0 matmuls
            return flush_b

        def emit_oproj(n, at_c):
            # Each core ships its raw o-proj partial (its 256 Wo rows);
            # the host sums the four TP partials per batch.  On-device
            # ReduceScatter has a ~20us latency floor per call that is
            # hard to hide, and the summation is 0.1% of the FLOPs.
            for m in range(4):
                obm = obuf.tile([P, H], BF16, name="ob")
                for nh in range(2):
                    po = pp.tile([P, CHW], F32, name="po", tag="pj")
                    for kt in range(2):
                        nc.tensor.matmul(
                            po[:], at_c[kt][:, m * P:(m + 1) * P],
                            wo_b[:, kt * H + nh * CHW:kt * H + (nh + 1) * CHW],
                            start=(kt == 0), stop=(kt == 1))
                    if nh == 0:
                        nc.vector.tensor_copy(obm[:, 0:CHW], po[:])
                    else:
                        nc.scalar.activation(obm[:, CHW:H], po[:], CPY)
                nc.gpsimd.dma_start(
                    out[n * CHW + m * P:n * CHW + (m + 1) * P, :], obm[:])

        emit_proj(0)
        for n in range(NCH):
            at_c = [attn_pool.tile([P, CHW], BF16, name=f"at_{kt}")
                    for kt in range(2)]
            fb = emit_attention(n, at_c)
            fb()
            if n + 1 < NCH:
                emit_proj(n + 1)
            emit_oproj(n, at_c)

    nc.compile()
    return nc


def _host_inputs(hidden_states, cos, sin, Wq, Wk, Wv, Wo):
    import ml_dtypes

    bf16 = ml_dtypes.bfloat16
    x = np.asarray(hidden_states, np.float32)
    cos = np.asarray(cos, np.float32)
    sin = np.asarray(sin, np.float32)
    Wq = np.asarray(Wq, np.float32)
    Wk = np.asarray(Wk, np.float32)
    Wv = np.asarray(Wv, np.float32)
    Wo = np.asarray(Wo, np.float32)

    ct2 = np.ascontiguousarray(np.tile(cos.T, (2, 1)))       # [128, S]
    st2 = np.ascontiguousarray(np.tile(sin.T, (2, 1)))
    r64 = np.zeros((64, 64), np.float32)
    for i in range(32):
        r64[32 + i, i] = -1.0
        r64[i, 32 + i] = 1.0
    rot = np.zeros((128, 128), np.float32)
    rot[0:64, 0:64] = r64
    rot[64:128, 64:128] = r64
    ident = np.eye(64, dtype=np.float32)
    tri = np.triu(np.ones((128, 128), np.float32))
    ones1 = np.ones((1, 64), np.float32)
    onescol = np.ones((128, 1), np.float32)

    def ktile(a, nt):
        # [nt*128, F] -> [128, nt, F] (partition-major interleave so one
        # DMA covers all nt contraction tiles)
        f = a.shape[1]
        return np.ascontiguousarray(
            a.reshape(nt, 128, f).transpose(1, 0, 2).astype(bf16))

    xTs = [ktile(x[d].T, 8) for d in range(B)]
    in_maps = []
    for c_id in range(NCORES):
        d, g = c_id // 4, c_id % 4
        in_maps.append({
            "xT": xTs[d],
            "wq": ktile(Wq[:, g * 256:(g + 1) * 256], 8),
            "wkv": ktile(
                np.concatenate([Wk[:, g * 64:(g + 1) * 64],
                                Wv[:, g * 64:(g + 1) * 64]], axis=1), 8),
            "wo": ktile(Wo[g * 256:(g + 1) * 256, :], 2),
            "ct2": ct2.astype(bf16), "st2": st2.astype(bf16),
            "rot": rot.astype(bf16), "ident": ident.astype(bf16),
            "tri": tri.astype(bf16), "ones1": ones1.astype(bf16),
            "onescol": onescol.astype(bf16),
        })
    return in_maps


def _assemble(results):
    full = np.empty((B, S, H), np.float32)
    for d in range(B):
        acc = np.zeros((S, H), np.float32)
        for g in range(4):
            acc += np.asarray(results[d * 4 + g]["out"]).astype(np.float32)
        full[d] = acc
    return full


def kernel(hidden_states, cos, sin, attention_mask, Wq, Wk, Wv, Wo):
    from concourse.bass_utils import run_bass_kernel_spmd
    if "nc" not in _prog_cache:
        _prog_cache["nc"] = _build()
    nc = _prog_cache["nc"]
    in_maps = _host_inputs(hidden_states, cos, sin, Wq, Wk, Wv, Wo)
    res = run_bass_kernel_spmd(nc, in_maps, list(range(NCORES)))
    return _assemble(res.results)
